# revision 4
# baseline (speedup 1.0000x reference)
"""NURBS surface evaluation on 8 Trainium2 NeuronCores.

Math: the reference computes, for output grid point (e, f) and channel d,
    surf[e, f, d] = sum_{l,r} bx[e,l] * by[f,r] * P[ix[l,e], iy[r,f], d]
which factorizes exactly as a pair of matmuls against sparse basis-scatter
matrices built on the host from the (tiny) knot vectors:
    BxD[e, ix[l,e]] = bx[e,l]      (1024, 256)
    ByD[iy[r,f], f] = by[f,r]      (256, 1024)
    surf[:, :, d]   = BxD @ P[:, :, d] @ ByD

Sharding: the e (Ex) axis is split across 8 cores (128 rows each) -- fully
data parallel. Each core computes
    T1T_d[j, e_loc] = sum_i P[i, j, d] * BxT[i, e_loc]   (stage 1)
    out_d[e_loc, f] = sum_j T1T_d[j, e_loc] * ByD[j, f]  (stage 2)
Stage 1 contracts only over the core's control-row footprint window (spans
are monotone in e, so the window is ~40 rows; padded to 128 with zeros).
The per-core output is written as (3, 128, 1024) d-major planes and the
host interleaves the final (1, 1024, 1024, 3).
"""

import os

import numpy as np

DEGREE = 3
OUT_XY = 1024
N_CTRL = 256
EPS = 1e-05
N_CORES = 8
EC = OUT_XY // N_CORES  # output rows per core

# Set by kernel() on each call: BassKernelResults of the last device run
# (test harnessing only; carries exec_time_ns when tracing is enabled).
last_results = None


# ----------------------------------------------------------------------------
# Host-side prep: knots, spans, Cox-de Boor basis, scatter matrices
# ----------------------------------------------------------------------------

def _normalize_knots_np(k):
    k = np.where(k < 0.0, np.float32(1e-4), k.astype(np.float32))
    k = np.cumsum(k, dtype=np.float32)
    return ((k - k[0]) / (k[-1] - k[0])).astype(np.float32)


def _prep_scalars(knot_x_row, knot_y_row):
    """Normalized knot vectors and the eval grid.

    cumsum/linspace rounding depends on the backend; run these two tiny ops
    through jax-on-CPU when available so the values match the jax reference
    bit-for-bit. Everything downstream (searchsorted, basis arithmetic) is
    elementwise IEEE fp32 and matches numpy exactly.
    """
    try:
        import jax
        import jax.numpy as jnp

        cpu = jax.devices("cpu")[0]
        with jax.default_device(cpu):
            def nk(k):
                k = jnp.where(k < 0.0, jnp.asarray(1e-4, k.dtype), k)
                k = jnp.cumsum(k)
                return (k - k[0]) / (k[-1] - k[0])

            kx = np.asarray(nk(jnp.asarray(knot_x_row)))
            ky = np.asarray(nk(jnp.asarray(knot_y_row)))
            ev = np.asarray(jnp.linspace(EPS, 1.0 - EPS, OUT_XY, dtype=jnp.float32))
        return (kx.astype(np.float32), ky.astype(np.float32),
                ev.astype(np.float32))
    except Exception:
        ev = np.linspace(EPS, 1.0 - EPS, OUT_XY).astype(np.float32)
        return _normalize_knots_np(knot_x_row), _normalize_knots_np(knot_y_row), ev


def _find_spans(u, knots):
    spans = np.searchsorted(knots, u, side="right") - 1
    return np.where(u == knots[N_CTRL], N_CTRL - 1, spans)


def _basis(u, knots, span):
    # Cox-de Boor recursion, literal port of the reference (fp32 throughout).
    K = knots.shape[0]
    cols = [np.ones_like(u)]
    left = [None]
    right = [None]
    for j in range(1, DEGREE + 1):
        left.append(u - knots[np.mod(span + 1 - j, K)])
        right.append(knots[np.mod(span + j, K)] - u)
        saved = np.zeros_like(u)
        new_cols = []
        for r in range(j):
            temp = cols[r] / (right[r + 1] + left[j - r])
            new_cols.append(saved + right[r + 1] * temp)
            saved = left[j - r] * temp
        new_cols.append(saved)
        cols = new_cols
    return np.stack(cols, axis=-1)  # (E, DEGREE+1)


def _host_pack(control_points, knot_vector_x, knot_vector_y):
    P = np.ascontiguousarray(np.asarray(control_points, dtype=np.float32))
    kx, ky, ev = _prep_scalars(np.asarray(knot_vector_x, np.float32)[0],
                               np.asarray(knot_vector_y, np.float32)[0])
    sx = _find_spans(ev, kx)
    sy = _find_spans(ev, ky)
    bx = _basis(ev, kx, sx).astype(np.float32)  # (1024, 4)
    by = _basis(ev, ky, sy).astype(np.float32)
    ixs = np.mod(sx[None, :] - DEGREE + np.arange(DEGREE + 1)[:, None], N_CTRL)
    iys = np.mod(sy[None, :] - DEGREE + np.arange(DEGREE + 1)[:, None], N_CTRL)

    BxD = np.zeros((OUT_XY, N_CTRL), np.float32)
    BxD[np.arange(OUT_XY)[:, None], ixs.T] = bx
    ByD = np.zeros((N_CTRL, OUT_XY), np.float32)
    ByD[iys, np.arange(OUT_XY)[None, :]] = by.T

    # Per-core stage-1 footprint windows (rows of P actually touched).
    los, widths = [], []
    for c in range(N_CORES):
        s = sx[EC * c:EC * (c + 1)]
        lo = int(s.min()) - DEGREE
        w = int(s.max()) - lo + 1
        if w > N_CTRL:  # degenerate: full wrap; use the identity window
            lo, w = 0, N_CTRL
        los.append(lo)
        widths.append(w)
    k1 = 128 if max(widths) <= 128 else N_CTRL

    p_loc = np.zeros((N_CORES, k1, N_CTRL, 3), np.float32)
    bxt = np.zeros((N_CORES, k1, EC), np.float32)
    for c in range(N_CORES):
        rows = (los[c] + np.arange(widths[c])) % N_CTRL
        p_loc[c, :widths[c]] = P[rows]
        bxt[c, :widths[c]] = BxD[EC * c:EC * (c + 1)][:, rows].T
    return p_loc, bxt, ByD, k1


# ----------------------------------------------------------------------------
# Device kernel
# ----------------------------------------------------------------------------

def _split_multi_waits(nc):
    """Hoist extra semaphore waits onto standalone NoOps.

    The walrus build in this container rejects any instruction carrying more
    than one SyncWait ("Too many sync wait commands"), but Tile emits the
    full wait set on the consuming instruction. Splitting them into
    preceding single-wait NoOps on the same engine stream is semantically
    identical (the engine stalls at each wait in order).
    """
    import concourse.mybir as mybir

    for fn in nc.m.functions:
        for blk in fn.blocks:
            new_insts = []
            for inst in blk.instructions:
                si = getattr(inst, "sync_info", None)
                if si is not None and si.on_wait and len(si.on_wait) > 1:
                    waits = list(si.on_wait)
                    for w in waits[:-1]:
                        new_insts.append(mybir.InstNoOp(
                            name=nc.get_next_instruction_name(),
                            sync_info=mybir.SyncInfo(on_wait=[w], on_update=[]),
                            bass_nofuse=True,
                            engine=inst.engine,
                        ))
                    inst.sync_info = mybir.SyncInfo(
                        on_wait=[waits[-1]], on_update=list(si.on_update))
                new_insts.append(inst)
            blk.instructions = new_insts
    return nc


def _build_bass(k1):
    import concourse.bass as bass
    import concourse.mybir as mybir
    from concourse.tile import TileContext

    f32 = mybir.dt.float32
    nc = bass.Bass()
    p_in = nc.dram_tensor("p_loc", [k1, N_CTRL, 3], f32, kind="ExternalInput")
    bxt_in = nc.dram_tensor("bxt", [k1, EC], f32, kind="ExternalInput")
    byd_in = nc.dram_tensor("byd", [N_CTRL, OUT_XY], f32, kind="ExternalInput")
    out_t = nc.dram_tensor("out", [3, EC, OUT_XY], f32, kind="ExternalOutput")

    nk = k1 // 128
    with TileContext(nc) as tc:
        with tc.tile_pool(name="const", bufs=1) as cpool, \
             tc.tile_pool(name="ps1", bufs=3, space="PSUM") as ps1pool, \
             tc.tile_pool(name="ps2", bufs=4, space="PSUM") as ps2pool:
            p_sb, bxt_sb = [], []
            for kc in range(nk):
                pt = cpool.tile([128, N_CTRL, 3], f32, tag=f"p{kc}", name=f"p{kc}")
                nc.sync.dma_start(out=pt[:], in_=p_in[kc * 128:(kc + 1) * 128])
                p_sb.append(pt)
                bt = cpool.tile([128, EC], f32, tag=f"bxt{kc}", name=f"bxt{kc}")
                nc.sync.dma_start(out=bt[:], in_=bxt_in[kc * 128:(kc + 1) * 128])
                bxt_sb.append(bt)
            byd_sb = []
            for jt in range(2):
                yt = cpool.tile([128, OUT_XY], f32, tag=f"byd{jt}", name=f"byd{jt}")
                nc.sync.dma_start(out=yt[:], in_=byd_in[jt * 128:(jt + 1) * 128])
                byd_sb.append(yt)

            # Stage 1: T1T_d[j, e] = sum_i P[i, j, d] * BxT[i, e]
            t1t_sb = [cpool.tile([128, 3, EC], f32, tag=f"t1t{jt}", name=f"t1t{jt}")
                      for jt in range(2)]
            for jt in range(2):
                for d in range(3):
                    ps = ps1pool.tile([128, EC], f32, tag="ps1", name="ps1")
                    for kc in range(nk):
                        nc.tensor.matmul(
                            ps[:],
                            lhsT=p_sb[kc][:, jt * 128:(jt + 1) * 128, d],
                            rhs=bxt_sb[kc][:],
                            start=(kc == 0),
                            stop=(kc == nk - 1),
                        )
                    nc.vector.tensor_copy(out=t1t_sb[jt][:, d], in_=ps[:])

            # Stage 2: out_d[e, f] = sum_j T1T_d[j, e] * ByD[j, f]
            out_sb = cpool.tile([128, 3, OUT_XY], f32, tag="out", name="out_sb")
            for d in range(3):
                for fc in range(2):
                    ps2 = ps2pool.tile([128, 512], f32, tag="ps2", name="ps2")
                    for jt in range(2):
                        nc.tensor.matmul(
                            ps2[:],
                            lhsT=t1t_sb[jt][:, d],
                            rhs=byd_sb[jt][:, fc * 512:(fc + 1) * 512],
                            start=(jt == 0),
                            stop=(jt == 1),
                        )
                    dst = out_sb[:, d, fc * 512:(fc + 1) * 512]
                    if (2 * d + fc) % 2 == 0:
                        nc.vector.tensor_copy(out=dst, in_=ps2[:])
                    else:
                        nc.scalar.copy(out=dst, in_=ps2[:])
            for d in range(3):
                nc.sync.dma_start(out=out_t[d], in_=out_sb[:, d])
    return nc


def kernel(control_points, knot_vector_x, knot_vector_y):
    global last_results
    from concourse.bass_utils import run_bass_kernel_spmd

    p_loc, bxt, byd, k1 = _host_pack(control_points, knot_vector_x,
                                     knot_vector_y)
    nc = _split_multi_waits(_build_bass(k1))
    in_maps = [{"p_loc": p_loc[c], "bxt": bxt[c], "byd": byd}
               for c in range(N_CORES)]
    trace = bool(int(os.environ.get("NURBS_TRACE", "0")))
    res = run_bass_kernel_spmd(nc, in_maps, core_ids=list(range(N_CORES)),
                               trace=trace)
    last_results = res
    full = np.empty((1, OUT_XY, OUT_XY, 3), np.float32)
    for c in range(N_CORES):
        full[0, EC * c:EC * (c + 1)] = res.results[c]["out"].transpose(1, 2, 0)
    return full
